# revision 1
# baseline (speedup 1.0000x reference)
"""MlpAttentionLayer Trainium2 kernel.

Math (reference):
  cat = [x, x-q, q]                         [B,T,3D]
  h   = BN1(cat); p = relu(h @ W1)          [B,T,D]
  g   = BN2(p);   w = sigmoid(g @ W2)       [B,T,1]
  out = sum_t x * w                         [B,D]

BN1 is affine per-feature, so with s1 = g1/sqrt(v1+eps):
  p_pre = x @ Wx + q @ Wq + bias0
    Wx    = s1a*W1a + s1b*W1b           (per-row scaled, [D,D])
    Wq    = s1c*W1c - s1b*W1b           ([D,D])
    bias0 = (b1 - m1*s1) @ W1           ([D])
BN2+W2 fold to:  logits = relu(p_pre) @ W2p + c2,  W2p = s2*W2, c2 scalar.

Host precomputes Wx, Qp = q@Wq + bias0, W2p, c2. Device (per core, 256 batch):
  xT via SWDGE-free path: HWDGE fp32 load -> on-chip cast to bf16 ->
  HWDGE xbar dma transpose.  preT = Wx^T @ xT  (+ Qp via one-hot matmul),
  relu -> h1T, logits via col-tiled M=32 matmuls, batched sigmoid,
  PE transpose of w-rows, final out_b = x_b^T @ w_b as per-b matvec, PSUM
  column-packed; one transpose at the end for the [B,D] output layout.
"""

import sys

sys.path.insert(0, "/opt/trn_rl_repo")

import numpy as np
import ml_dtypes

BN_EPS = 1e-3
B, T, D = 2048, 200, 128
N_CORES = 8
BSH = B // N_CORES          # 256 batch elements per core
G = 4                       # batch elements per pipeline group
NGRP = BSH // G             # 64 groups
TA, TBV = 112, 88           # t-tile split 200 = 112 + 88 (both loads)
TBP = 96                    # padded partition count of second t-tile (%16)
TP = TA + TBP               # 208 padded token count
XT_STRIDE = 224             # xT free stride per b (32B-aligned chunks)
PRE_STRIDE = 512            # per-b stride in pre: one full PSUM bank per accumulation group

BF16 = ml_dtypes.bfloat16
DEBUG = False


def _build_bass():
    from concourse import bacc, mybir
    from concourse.tile import TileContext
    from concourse.masks import make_identity

    fp32 = mybir.dt.float32
    bf16 = mybir.dt.bfloat16
    AF = mybir.ActivationFunctionType

    # Bacc (not bare Bass): its compile() legalizes multi-wait instructions
    # (walrus accepts at most one embedded sync wait per instruction).
    nc = bacc.Bacc()
    if DEBUG:
        dbg_h1 = nc.dram_tensor("dbg_h1", (128, G * TP), bf16, kind="ExternalOutput")
        dbg_w = nc.dram_tensor("dbg_w", (128, TP), fp32, kind="ExternalOutput")
        dbg_xt = nc.dram_tensor("dbg_xt", (128, G, XT_STRIDE), bf16, kind="ExternalOutput")
    x_d = nc.dram_tensor("x", (BSH, T, D), fp32, kind="ExternalInput")
    qp_d = nc.dram_tensor("qp", (BSH, D), bf16, kind="ExternalInput")
    wx_d = nc.dram_tensor("wx", (D, D), bf16, kind="ExternalInput")
    w2r_d = nc.dram_tensor("w2r", (D, 32), bf16, kind="ExternalInput")
    c2_d = nc.dram_tensor("c2", (1, 1), fp32, kind="ExternalInput")
    out_d = nc.dram_tensor("out", (BSH, D), fp32, kind="ExternalOutput")

    with TileContext(nc) as tc:
        with (
            tc.tile_pool(name="const", bufs=1) as cpool,
            tc.tile_pool(name="xin", bufs=3) as xpool,
            tc.tile_pool(name="x16", bufs=3) as x16pool,
            tc.tile_pool(name="xt", bufs=3) as xtpool,
            tc.tile_pool(name="mid", bufs=2) as midpool,
            tc.tile_pool(name="fin", bufs=1) as finpool,
            tc.tile_pool(name="ps_pre", bufs=1, space="PSUM") as pre_pool,
            tc.tile_pool(name="ps_xt", bufs=2, space="PSUM") as xt_pool,
            tc.tile_pool(name="ps_lw", bufs=1, space="PSUM") as lw_pool,
            tc.tile_pool(name="ps_out", bufs=1, space="PSUM") as fout_pool,
        ):
            ident16 = cpool.tile([128, 128], bf16)
            make_identity(nc, ident16)
            ident32 = cpool.tile([128, 128], fp32)
            make_identity(nc, ident32)
            wx_sb = cpool.tile([D, D], bf16)
            nc.sync.dma_start(wx_sb, wx_d[:, :])
            w2r_sb = cpool.tile([D, 32], bf16)
            nc.sync.dma_start(w2r_sb, w2r_d[:, :])
            c2_sb = cpool.tile([128, 1], fp32)
            nc.sync.dma_start(c2_sb, c2_d[0, 0:1].broadcast_to((128, 1)))
            # Qp slabs: [K=128 b-slots, M=128] stationary for the one-hot add
            qp_sb = cpool.tile([128, 2, D], bf16)
            nc.sync.dma_start(
                qp_sb, qp_d[:, :].rearrange("(s k) d -> k s d", k=128)
            )

            fout = fout_pool.tile([128, BSH], mybir.dt.float32)

            for gi in range(NGRP):
                b0 = gi * G
                # ---- load fp32 natural tiles, [t-part, b, d] layout
                x32a = xpool.tile([TA, G, D], fp32, tag="x32a")
                nc.sync.dma_start(
                    x32a, x_d[b0 : b0 + G, 0:TA, :].rearrange("b t d -> t b d")
                )
                x32b = xpool.tile([TBV, G, D], fp32, tag="x32b")
                nc.sync.dma_start(
                    x32b, x_d[b0 : b0 + G, TA:T, :].rearrange("b t d -> t b d")
                )
                # ---- cast to bf16 (DVE 2x single-src mode)
                x16a = x16pool.tile([TA, G, D], bf16, tag="x16a")
                nc.vector.tensor_copy(
                    x16a.rearrange("p b d -> p (b d)"),
                    x32a.rearrange("p b d -> p (b d)"),
                )
                x16b = x16pool.tile([TBP, G, D], bf16, tag="x16b")
                nc.vector.tensor_copy(
                    x16b[0:TBV].rearrange("p b d -> p (b d)"),
                    x32b.rearrange("p b d -> p (b d)"),
                )
                # ---- xT via PE transposes (bf16, FWL), pair-packed PSUM
                xt = xtpool.tile([128, G, XT_STRIDE], bf16, tag="xt")
                for pair in range(G // 2):
                    xtp = xt_pool.tile([128, 2 * XT_STRIDE], bf16, tag="xtp")
                    for gg in range(2):
                        g = 2 * pair + gg
                        c0 = gg * XT_STRIDE
                        nc.tensor.transpose(
                            xtp[:, c0 : c0 + TA], x16a[:, g, :], ident16[0:TA, 0:TA]
                        )
                        nc.tensor.transpose(
                            xtp[:, c0 + TA : c0 + TP],
                            x16b[:, g, :],
                            ident16[0:TBP, 0:TBP],
                        )
                    # one strided copy moves both b's xT to SBUF
                    if pair % 2 == 0:
                        nc.vector.tensor_copy(
                            xt[:, 2 * pair : 2 * pair + 2, 0:TP],
                            xtp.rearrange("p (b c) -> p b c", c=XT_STRIDE)[
                                :, :, 0:TP
                            ],
                        )
                    else:
                        nc.scalar.activation(
                            xt[:, 2 * pair : 2 * pair + 2, 0:TP],
                            xtp.rearrange("p (b c) -> p b c", c=XT_STRIDE)[
                                :, :, 0:TP
                            ],
                            AF.Copy,
                        )
                # ---- preT = Wx^T @ xT ; += Qp one-hot
                pre = pre_pool.tile([128, G * PRE_STRIDE], mybir.dt.float32)
                for g in range(G):
                    nc.tensor.matmul(
                        pre[:, g * PRE_STRIDE : g * PRE_STRIDE + TP],
                        wx_sb,
                        xt[:, g, 0:TP],
                        start=True,
                        stop=False,
                    )
                slab = (b0 // 128) % 2
                for g in range(G):
                    k = (b0 + g) % 128
                    nc.tensor.matmul(
                        pre[:, g * PRE_STRIDE : g * PRE_STRIDE + TP],
                        qp_sb[:, slab, :],
                        ident16[:, k : k + 1].broadcast_to((128, TP)),
                        start=False,
                        stop=True,
                    )
                # ---- relu -> h1T (alternate ACT/DVE)
                h1 = midpool.tile([128, G * TP], bf16, tag="h1")
                pre_v = pre.rearrange("p (g c) -> p g c", c=PRE_STRIDE)[:, :, 0:TP]
                h1_v = h1.rearrange("p (g c) -> p g c", c=TP)
                if gi % 2 == 0:
                    nc.scalar.activation(h1_v, pre_v, AF.Relu)
                else:
                    nc.vector.tensor_scalar_max(h1_v, pre_v, 0.0)
                # ---- logits: col-tiled M=32 matmuls into one PSUM tile
                lw = lw_pool.tile([128, 512], mybir.dt.float32)
                for g in range(G):
                    nc.tensor.matmul(
                        lw[32 * g : 32 * g + 32, 0:TP],
                        w2r_sb,
                        h1[:, g * TP : (g + 1) * TP],
                        start=True,
                        stop=True,
                        tile_position=(0, 32 * g),
                    )
                # ---- sigmoid (batched over the 4 b's on partitions)
                wsb = midpool.tile([128, TP], fp32, tag="wsb")
                nc.scalar.activation(
                    wsb, lw[:, 0:TP], AF.Sigmoid, bias=c2_sb[:, 0:1]
                )
                # ---- wT: PE transpose of w rows -> [t-part, w-cols]
                nc.tensor.transpose(lw[0:TA, 256 : 256 + 128], wsb[:, 0:TA], ident32)
                nc.tensor.transpose(
                    lw[0:TBP, 384 : 384 + 128], wsb[:, TA:TP], ident32
                )
                if DEBUG and gi == 0:
                    nc.sync.dma_start(dbg_h1[:, :], h1)
                    nc.sync.dma_start(dbg_w[:, :], wsb)
                    nc.sync.dma_start(dbg_xt[:, :, :], xt)
                wta = midpool.tile([TA, G], bf16, tag="wta")
                nc.vector.tensor_copy(
                    wta, bass_strided_cols(lw, 0, TA, 256, G)
                )
                wtb = midpool.tile([TBP, G], bf16, tag="wtb")
                nc.vector.tensor_copy(
                    wtb, bass_strided_cols(lw, 0, TBP, 384, G)
                )
                # ---- final: out_b = x_b^T @ w_b  (column-packed PSUM)
                for g in range(G):
                    bc = b0 + g
                    nc.tensor.matmul(
                        fout[:, bc : bc + 1],
                        x16a[:, g, :],
                        wta[:, g : g + 1],
                        start=True,
                        stop=False,
                    )
                    nc.tensor.matmul(
                        fout[:, bc : bc + 1],
                        x16b[0:TBV, g, :],
                        wtb[0:TBV, g : g + 1],
                        start=False,
                        stop=True,
                    )

            # ---- epilogue: transpose [d, b] -> [b, d] and store
            osb = finpool.tile([128, BSH], mybir.dt.float32)
            nc.scalar.activation(osb, fout, AF.Copy)
            obt = finpool.tile([128, BSH], mybir.dt.float32)
            for half in range(2):
                ot = lw_pool.tile([128, 512], mybir.dt.float32, tag="lw")
                nc.tensor.transpose(
                    ot[:, 0:128], osb[:, half * 128 : half * 128 + 128], ident32
                )
                nc.scalar.activation(
                    obt[:, half * 128 : half * 128 + 128], ot[:, 0:128], AF.Copy
                )
                nc.sync.dma_start(
                    out_d[half * 128 : half * 128 + 128, :],
                    obt[:, half * 128 : half * 128 + 128],
                )
    nc.finalize()
    return nc


def bass_strided_cols(tile, p0, pn, c0, n):
    """AP selecting columns c0, c0+32, ... (n of them) on partitions p0:p0+pn."""
    ap = tile[p0 : p0 + pn, c0 : c0 + 32 * n]
    return ap.rearrange("p (n c) -> p n c", c=32)[:, :, 0]


_NC_CACHE = {}


def _get_nc():
    if "nc" not in _NC_CACHE:
        _NC_CACHE["nc"] = _build_bass()
    return _NC_CACHE["nc"]


def kernel(
    inputs,
    query,
    W1,
    W2,
    bn1_gamma,
    bn1_beta,
    bn1_mean,
    bn1_var,
    bn2_gamma,
    bn2_beta,
    bn2_mean,
    bn2_var,
):
    from concourse.bass_utils import run_bass_kernel_spmd

    x = np.asarray(inputs, np.float32)
    q = np.asarray(query, np.float64)
    W1 = np.asarray(W1, np.float64)
    W2 = np.asarray(W2, np.float64)
    s1 = np.asarray(bn1_gamma, np.float64) / np.sqrt(
        np.asarray(bn1_var, np.float64) + BN_EPS
    )
    W1s = s1[:, None] * W1                       # scale rows of W1
    Wx = W1s[0:D] + W1s[D : 2 * D]               # [D, D]
    Wq = W1s[2 * D : 3 * D] - W1s[D : 2 * D]     # [D, D]
    bias0 = (np.asarray(bn1_beta, np.float64) - np.asarray(bn1_mean, np.float64) * s1) @ W1
    Qp = q @ Wq + bias0                          # [B, D]
    s2 = np.asarray(bn2_gamma, np.float64) / np.sqrt(
        np.asarray(bn2_var, np.float64) + BN_EPS
    )
    W2p = s2 * W2[:, 0]                          # [D]
    c2 = float(
        (np.asarray(bn2_beta, np.float64) - np.asarray(bn2_mean, np.float64) * s2)
        @ W2[:, 0]
    )

    wx16 = Wx.astype(BF16)                       # lhsT [K=din, M=dout]
    w2r16 = np.repeat(W2p.astype(BF16)[:, None], 32, axis=1)  # [D, 32]
    qp16 = Qp.astype(BF16)
    c2a = np.full((1, 1), c2, np.float32)

    nc = _get_nc()
    in_maps = []
    for c in range(N_CORES):
        in_maps.append(
            {
                "x": x[c * BSH : (c + 1) * BSH],
                "qp": qp16[c * BSH : (c + 1) * BSH],
                "wx": wx16,
                "w2r": w2r16,
                "c2": c2a,
            }
        )
    res = run_bass_kernel_spmd(nc, in_maps, core_ids=list(range(N_CORES)))
    out = np.concatenate([r["out"] for r in res.results], axis=0)
    return out.astype(np.float32)



# revision 18
# speedup vs baseline: 1.0251x; 1.0251x over previous
"""MlpAttentionLayer Trainium2 kernel (v2 — transpose-free, DVE matvec).

Math (reference):
  cat = [x, x-q, q]                         [B,T,3D]
  h   = BN1(cat); p = relu(h @ W1)          [B,T,D]
  g   = BN2(p);   w = sigmoid(g @ W2)       [B,T,1]
  out = sum_t x * w                         [B,D]

BN1 is affine per-feature, so with s1 = g1/sqrt(v1+eps):
  p_pre = x @ Wx + Qp[b],  Wx = s1a*W1a + s1b*W1b,  Qp = q @ Wq + bias0
BN2+W2 fold to:  logits = relu(p_pre) @ W2p + c2,  c2 scalar.

Device plan (per core, 256 batch rows):
  Host pre-transposes x to bf16 xT [D, BSH, T] so the main matmul's moving
  operand loads directly (1.6 KB contiguous DMA runs, no on-chip transposes).
  preT = Wx^T @ xT (PE, Wx stationary).  relu+Qp bias fused in one
  tensor_scalar(add qp, max 0) per b, rotated across DVE/GPSIMD.
  logits use a 128-replicated W2p stationary so sigmoid output is broadcast
  across all partitions; the final weighted sum runs as fused
  tensor_tensor_reduce on DVE: outT[:,b] = sum_t xT[:,t]*sig[:,t].
  Epilogue: two PE transposes [d,b] -> [b,d] and store.
"""

import sys

sys.path.insert(0, "/opt/trn_rl_repo")

import numpy as np
import ml_dtypes

BN_EPS = 1e-3
B, T, D = 2048, 200, 128
N_CORES = 8
BSH = B // N_CORES          # 256 batch elements per core
G = 4                       # batch elements per pipeline group
NGRP = BSH // G             # 64 groups
PRES = 512                  # per-b PSUM stride (fp32 elems): one 2KB bank per b
                            # (accumulation groups for the 4 b's are open
                            # concurrently; each must own its zero region)

BF16 = ml_dtypes.bfloat16
DEBUG = False

# relu engine rotation per group: v=DVE, a=ACT (GPSIMD cannot read PSUM)
RELU_PATTERN = "vvvaa"


def _build_bass():
    from concourse import bacc, mybir
    from concourse.tile import TileContext
    from concourse.masks import make_identity

    fp32 = mybir.dt.float32
    bf16 = mybir.dt.bfloat16
    AF = mybir.ActivationFunctionType
    OP = mybir.AluOpType

    nc = bacc.Bacc()
    xt_d = nc.dram_tensor("xt", (D, BSH, T), bf16, kind="ExternalInput")
    qp_d = nc.dram_tensor("qp", (BSH, D), bf16, kind="ExternalInput")
    wx_d = nc.dram_tensor("wx", (D, D), bf16, kind="ExternalInput")
    w2r_d = nc.dram_tensor("w2r", (D, 128), bf16, kind="ExternalInput")
    c2_d = nc.dram_tensor("c2", (1, 1), fp32, kind="ExternalInput")
    out_d = nc.dram_tensor("out", (BSH, D), fp32, kind="ExternalOutput")

    with TileContext(nc) as tc:
        with (
            tc.tile_pool(name="const", bufs=1) as cpool,
            tc.tile_pool(name="xt", bufs=4) as xtpool,
            tc.tile_pool(name="h1", bufs=2) as h1pool,
            tc.tile_pool(name="sig", bufs=2) as sigpool,
            tc.tile_pool(name="scr", bufs=2) as scrpool,
            tc.tile_pool(name="fin", bufs=1) as finpool,
            tc.tile_pool(name="ps_pre", bufs=2, space="PSUM") as prepool,
        ):
            ident32 = cpool.tile([128, 128], fp32)
            make_identity(nc, ident32)
            ident16 = cpool.tile([128, 128], bf16)
            make_identity(nc, ident16)
            wx_sb = cpool.tile([D, D], bf16)
            nc.sync.dma_start(wx_sb, wx_d[:, :])
            w2r_sb = cpool.tile([D, 128], bf16)
            nc.sync.dma_start(w2r_sb, w2r_d[:, :])
            c2_sb = cpool.tile([128, 1], fp32)
            nc.sync.dma_start(c2_sb, c2_d[0, 0:1].broadcast_to((128, 1)))
            # Qp slabs: [K=128 b-slots, M=128] stationary for the one-hot add
            qp_sb = cpool.tile([128, 2, D], bf16)
            nc.sync.dma_start(
                qp_sb, qp_d[:, :].rearrange("(s k) d -> k s d", k=128)
            )

            outT = finpool.tile([128, BSH], fp32)

            relu_engines = {
                "v": nc.vector,
                "a": nc.scalar,
            }

            for gi in range(NGRP):
                b0 = gi * G
                xt = xtpool.tile([D, G, T], bf16, tag="xt")
                nc.sync.dma_start(xt, xt_d[:, b0 : b0 + G, :])
                # ---- preT = Wx^T @ xT  (Wx stationary)
                pre = prepool.tile([128, G * PRES], fp32, tag="pre")
                pre_v = pre.rearrange("p (g c) -> p g c", c=PRES)
                for g in range(G):
                    nc.tensor.matmul(
                        pre_v[:, g, 0:T], wx_sb, xt[:, g, :],
                        start=True, stop=False,
                    )
                # ---- += Qp via one-hot matmul (PE has headroom)
                slab = (b0 // 128) % 2
                for g in range(G):
                    k = (b0 + g) % 128
                    nc.tensor.matmul(
                        pre_v[:, g, 0:T],
                        qp_sb[:, slab, :],
                        ident16[:, k : k + 1].broadcast_to((128, T)),
                        start=False, stop=True,
                    )
                # ---- h1 = relu(pre)  (one batched instr, rotated DVE/ACT)
                h1 = h1pool.tile([128, G, T], bf16, tag="h1")
                which = RELU_PATTERN[gi % len(RELU_PATTERN)]
                if which == "a":
                    nc.scalar.activation(h1, pre_v[:, :, 0:T], AF.Relu)
                else:
                    nc.vector.tensor_scalar_max(h1, pre_v[:, :, 0:T], 0.0)
                # ---- logits (128-replicated stationary -> broadcast rows).
                # Reuses the pre tile's banks: relu has consumed pre by now,
                # and the WAR dependency matches the h1 data dependency.
                lw_v = pre_v
                for g in range(G):
                    nc.tensor.matmul(
                        lw_v[:, g, 0:T], w2r_sb, h1[:, g, :],
                        start=True, stop=True,
                    )
                # ---- sigmoid (batched over 4 b's), broadcast layout
                sg = sigpool.tile([128, G, T], bf16, tag="sg")
                nc.scalar.activation(
                    sg, lw_v[:, :, 0:T], AF.Sigmoid, bias=c2_sb[:, 0:1]
                )
                # ---- outT[:, b] = sum_t xT[:, t] * sig[:, t]
                # (TENSOR_TENSOR_REDUCE is rejected by this runtime, so:
                #  DVE bf16 multiply, then reduce split DVE/GPSIMD)
                scr = scrpool.tile([128, G, T], bf16, tag="scr")
                scr2 = scrpool.tile([128, G, T], bf16, tag="scr2")
                for g in range(G):
                    eng = nc.gpsimd if g % 2 else nc.vector
                    eng.tensor_tensor(
                        scr[:, g, :], xt[:, g, :], sg[:, g, :], OP.mult
                    )
                for g in range(G):
                    nc.vector.tensor_scalar(
                        scr2[:, g, :], scr[:, g, :], 1.0, None,
                        OP.mult, OP.add,
                        accum_out=outT[:, b0 + g : b0 + g + 1],
                    )

            # ---- epilogue: transpose [d, b] -> [b, d] and store
            obt = finpool.tile([128, BSH], fp32)
            for half in range(2):
                ot = prepool.tile([128, G * PRES], fp32, tag="pre")
                nc.tensor.transpose(
                    ot[:, 0:128], outT[:, half * 128 : half * 128 + 128], ident32
                )
                nc.scalar.activation(
                    obt[:, half * 128 : half * 128 + 128], ot[:, 0:128], AF.Copy
                )
                nc.sync.dma_start(
                    out_d[half * 128 : half * 128 + 128, :],
                    obt[:, half * 128 : half * 128 + 128],
                )
    nc.finalize()
    return nc


_NC_CACHE = {}


def _get_nc():
    if "nc" not in _NC_CACHE:
        _NC_CACHE["nc"] = _build_bass()
    return _NC_CACHE["nc"]


def _prep_host(
    inputs, query, W1, W2,
    bn1_gamma, bn1_beta, bn1_mean, bn1_var,
    bn2_gamma, bn2_beta, bn2_mean, bn2_var,
):
    """Fold BN into weights, precompute Qp, pre-transpose x; returns in_maps."""
    x = np.asarray(inputs, np.float32)
    q = np.asarray(query, np.float64)
    W1 = np.asarray(W1, np.float64)
    W2 = np.asarray(W2, np.float64)
    s1 = np.asarray(bn1_gamma, np.float64) / np.sqrt(
        np.asarray(bn1_var, np.float64) + BN_EPS
    )
    W1s = s1[:, None] * W1
    Wx = W1s[0:D] + W1s[D : 2 * D]               # [D, D]
    Wq = W1s[2 * D : 3 * D] - W1s[D : 2 * D]     # [D, D]
    bias0 = (
        np.asarray(bn1_beta, np.float64) - np.asarray(bn1_mean, np.float64) * s1
    ) @ W1
    Qp = q @ Wq + bias0                          # [B, D]
    s2 = np.asarray(bn2_gamma, np.float64) / np.sqrt(
        np.asarray(bn2_var, np.float64) + BN_EPS
    )
    W2p = s2 * W2[:, 0]                          # [D]
    c2 = float(
        (np.asarray(bn2_beta, np.float64) - np.asarray(bn2_mean, np.float64) * s2)
        @ W2[:, 0]
    )

    wx16 = Wx.astype(BF16)                       # lhsT [K=din, M=dout]
    w2r16 = np.repeat(W2p.astype(BF16)[:, None], 128, axis=1)  # [D, 128]
    qp16 = Qp.astype(BF16)                       # [B, D]
    c2a = np.full((1, 1), c2, np.float32)

    x16 = x.astype(BF16)                         # [B, T, D]
    in_maps = []
    for c in range(N_CORES):
        xs = x16[c * BSH : (c + 1) * BSH]        # [BSH, T, D]
        xtc = np.ascontiguousarray(xs.transpose(2, 0, 1))  # [D, BSH, T]
        in_maps.append(
            {
                "xt": xtc,
                "qp": qp16[c * BSH : (c + 1) * BSH],
                "wx": wx16,
                "w2r": w2r16,
                "c2": c2a,
            }
        )
    return in_maps


def kernel(**inputs):
    from concourse.bass_utils import run_bass_kernel_spmd

    in_maps = _prep_host(**inputs)
    nc = _get_nc()
    res = run_bass_kernel_spmd(nc, in_maps, core_ids=list(range(N_CORES)))
    out = np.concatenate([r["out"] for r in res.results], axis=0)
    return out.astype(np.float32)
